# revision 27
# baseline (speedup 1.0000x reference)
"""Trainium2 Bass kernel for the pairwise-MLP geometric convolution.

Reference computes, per batch z:
    rel[a,b]   = g[b] - g[a]
    h[a,b,:]   = relu(rel @ W1 + b1)                      [N,N,H]
    k[a,b,:]   = h @ W2 + b2  -> [N,N,C_OUT,C_IN]
    out[a,i]   = sum_{b,j} k[a,b,i,j] * f[b,j]

Factorization (k never materialized):
    U = g @ W1
    T[(b,h), a] = relu(U[b,h] + b1[h] - U[a,h])
    G[(b,h), i] = sum_j W2[h, i*C_IN+j] * f[b,j]
    out[a, i]   = sum_{(b,h)} T[(b,h),a] * G[(b,h),i]
                + sum_j b2[i,j] * (sum_b f[b,j])

Sharding over 8 cores: z (2) x b-quarter (4); each core computes the full
[a=256, i=32] partial for its 64 b's; host sums quarters.

Per-core dataflow (all matmul operands bf16, PSUM accumulation fp32):
  * K-chunks of 128 = (b-pair bl in {0,1}) x (h=64). 32 pairs.
  * G lands DIRECTLY in [(bl,h), (p,i)] PSUM layout via 64 small matmuls
    (lhsT = W2 packed [j, h]-per-i on host, rhs = even/odd-b features,
    out partition-offset 64*bl, column stride 32) -- no reshape DMAs.
  * Ub+b1 lands directly as [(bl,h), p] via one delta-expanded matmul
    (lhsT rows (bl',x) = [W1;b1] on the bl=bl' diagonal, rhs = paired g).
  * T tiles [128, 256] built by DVE (bf16 2x tensor_scalar, ~127ns),
    ACT (activation reading U from PSUM with scale=-1), and GPSIMD.
  * Main chain: out[a_half, i] += t_p[:, half].T @ g_p  -- 32-row matmuls,
    T stationary, so the PE streams 2048 rows instead of 8192.
  * b2 bias via rank-1 matmuls (ones x (b2^T @ sum_b f)).

Hardware constraint honored throughout: the walrus codegen accepts at most
ONE sync-wait per TPB instruction; consumers observe multi-engine deps
through single-wait dummy ops (Pool copies, PE scrap matmul).
"""

import os
import sys

import numpy as np

_TRN_REPO = "/opt/trn_rl_repo"
if _TRN_REPO not in sys.path:
    sys.path.insert(0, _TRN_REPO)

from contextlib import ExitStack

import concourse.bass as bass
import concourse.mybir as mybir
import concourse.tile as tile
from concourse.bass_utils import run_bass_kernel_spmd

from concourse.vector_clock import ScopedClock

# The walrus codegen used on the axon/PJRT path accepts at most ONE sync-wait
# per TPB instruction. Tile's kernel-tail drain aggregates a wait for every
# live semaphore onto a single Drain, which walrus rejects. Patch the tail to
# spread those waits across single-wait SP nops before an unadorned drain.
_orig_drain_and_barrier = tile.TileContext._drain_and_barrier


def _split_wait_drain_and_barrier(self, tick_clock, wait_clock):
    nc = self.nc
    probe = nc.sync.nop(nofuse=True)
    wait_clock.add_sem_waits(probe.ins, ScopedClock({None: tick_clock.global_clock}))
    si = probe.ins.sync_info
    waits = list(si.on_wait) if si is not None and si.on_wait else []
    if len(waits) > 1:
        probe.ins.sync_info = mybir.SyncInfo(on_wait=waits[:1], on_update=[])
        for w in waits[1:]:
            extra = nc.sync.nop(nofuse=True)
            extra.ins.sync_info = mybir.SyncInfo(on_wait=[w], on_update=[])
    nc.sync.drain()
    nc.all_engine_barrier()
    popped = nc._tile_sem_poison_stack.pop()
    assert popped is self._sem_poison
    nc.clear_and_free_semaphores(list(self.sems.allocated().values()))
    nc.all_engine_barrier()


tile.TileContext._drain_and_barrier = _split_wait_drain_and_barrier

# The Bass constructor registers four const APs via gpsimd.memset; Pool's Q7
# launch overhead puts ~0.4us of serial work ahead of the opening all-engine
# barrier. Reroute those preamble memsets to the (faster, otherwise idle) DVE
# queue. The barrier after them still guarantees completion.
_orig_bass_init = bass.Bass.__init__


def _patched_bass_init(self, *a, **k):
    self._in_preamble_init = True
    try:
        _orig_bass_init(self, *a, **k)
    finally:
        self._in_preamble_init = False


_orig_memset = bass.BassEitherVectorEngine.memset


def _patched_memset(self, ap, constant):
    b = getattr(self, "bass", None)
    if b is not None and getattr(b, "_in_preamble_init", False):
        return _orig_memset(b.vector, ap, constant)
    return _orig_memset(self, ap, constant)


bass.Bass.__init__ = _patched_bass_init
bass.BassEitherVectorEngine.memset = _patched_memset

F32 = mybir.dt.float32
BF16 = mybir.dt.bfloat16
Z, N, C_IN, C_OUT, H = 2, 256, 32, 32, 64
BQ = 64          # b-points per core (N / 4 quarters)
NPAIR = BQ // 2  # 32 K-chunks of (2 b x 64 h) = 128

# d1 [32, 832]: rows 0:3 -> gT (cols 0:256), W1dup (256:384);
#               rows 0:8 -> gb8 (384:416), w1bexp (416:544);
#               rows 0:32 -> f2e (544:576), f2o (576:608), fTb (608:672),
#               b2t (672:704), ones row 0 (704:832)
D1_P, D1_W = 32, 832
# wexp [32, 2048]: [j, i*64+h] = W2[h, i*32+j]
WEXP_P, WEXP_W = 32, 2048

# T-build schedule: engine of each build slot in production order.
# v=DVE (127ns), s=ACT (398ns), g=GPSIMD (451ns). ACT also does the
# ubT2 copy first and the G PSUM->SBUF copy mid-stream.
N_V, N_S, N_G = 23, 3, 6
N_WARM = 8        # p-state warmers
N_TWARM = 0       # T-gated warmers between G matmuls and main chain


def _t_schedule():
    """Interleave T-builds across engines by estimated completion time.

    Returns a list of engine codes, one per pair, in estimated completion
    order (= PE consumption order).
    """
    slots = []
    for k in range(N_V):
        slots.append((4260 + 127 * (k + 1), "v"))
    for k in range(N_S):
        slots.append((5670 + 398 * (k + 1), "s"))
    for k in range(N_G):
        slots.append((4500 + 451 * (k + 1), "g"))
    slots.sort()
    return [e for _, e in slots]


def build_nc(debug: bool = False) -> bass.Bass:
    nc = bass.Bass("TRN2", target_bir_lowering=False, debug=debug, num_devices=8)

    d1 = nc.dram_tensor("d1", [D1_P, D1_W], BF16, kind="ExternalInput").ap()
    wexp = nc.dram_tensor("wexp", [WEXP_P, WEXP_W], BF16, kind="ExternalInput").ap()
    outp = nc.dram_tensor("outp", [128, 2 * C_OUT], BF16, kind="ExternalOutput").ap()

    sched = _t_schedule()

    with tile.TileContext(nc) as tc, ExitStack() as ctx:
        consts = ctx.enter_context(tc.tile_pool(name="consts", bufs=1))
        work = ctx.enter_context(tc.tile_pool(name="work", bufs=1))
        # every T tile gets its own slot so no T-op waits on a PE release
        tpool = ctx.enter_context(tc.tile_pool(name="tpool", bufs=NPAIR))
        psum = ctx.enter_context(tc.tile_pool(name="psum", bufs=1, space="PSUM"))

        # ---- input loads: both on the SP queue, d1 first
        d1_sb = consts.tile([D1_P, D1_W], BF16)
        nc.sync.dma_start(out=d1_sb, in_=d1)
        wexp_sb = consts.tile([WEXP_P, WEXP_W], BF16)
        nc.sync.dma_start(out=wexp_sb[:, 0:1024], in_=wexp[:, 0:1024])
        nc.sync.dma_start(out=wexp_sb[:, 1024:2048], in_=wexp[:, 1024:2048])

        gT = d1_sb[0:3, 0:256]
        w1dup = d1_sb[0:3, 256:384]
        gb8 = d1_sb[0:8, 384:416]
        w1bexp = d1_sb[0:8, 416:544]
        f2e = d1_sb[0:32, 544:576]
        f2o = d1_sb[0:32, 576:608]
        fTb = d1_sb[0:32, 608:672]
        b2t = d1_sb[0:32, 672:704]
        ones_row = d1_sb[0:1, 704:832]

        # ---- PE p-state warm-up: memset a row on Pool, then dummy matmuls
        wsrc = work.tile([1, 256], BF16)
        nc.vector.memset(wsrc, 1.0)
        # PSUM tiles are access-chained by the dep tracker: give every
        # independently-consumed producer its own tile.
        wk_ps = psum.tile([128, 512], F32, name="wk_ps")
        u_ps = psum.tile([128, 256], F32, name="u_ps")
        ubbr_ps = psum.tile([128, 64], F32, name="ubbr_ps")
        br_ps_t = psum.tile([1, C_OUT], F32, name="br_ps_t")
        warm_ps = wk_ps[0:1, 0:256]
        for _ in range(N_WARM):
            nc.tensor.matmul(warm_ps, lhsT=wsrc[0:1, 0:1], rhs=wsrc,
                             start=True, stop=True)

        # ---- U matmuls (dup'd over both partition halves)
        uaT2_ps = u_ps
        nc.tensor.matmul(uaT2_ps, lhsT=w1dup, rhs=gT, start=True, stop=True)
        ub_ps = ubbr_ps[:, 0:NPAIR]
        nc.tensor.matmul(ub_ps, lhsT=w1bexp, rhs=gb8, start=True, stop=True)

        # ---- ubT2 (Ub + b1 in [(bl,h), p] layout) to SBUF on ACT
        ubT2 = work.tile([2 * H, NPAIR], F32)
        nc.scalar.activation(ubT2, ub_ps, mybir.ActivationFunctionType.Copy)

        # ---- small DVE chain: scol reduce, then negua2, then birow copy
        scol = work.tile([C_IN, 1], BF16)
        with nc.allow_low_precision(reason="bf16 matmul operand; one rounding"):
            nc.vector.tensor_reduce(out=scol, in_=fTb,
                                    axis=mybir.AxisListType.X,
                                    op=mybir.AluOpType.add)
        negua2 = work.tile([2 * H, N], BF16)
        nc.vector.tensor_scalar(out=negua2, in0=uaT2_ps, scalar1=-1.0,
                                scalar2=None, op0=mybir.AluOpType.mult)

        # b2 bias row: br = scol^T-contracted b2t  (biasrow mm waits DVE>=scol,
        # which also subsumes the d1b DMA for every later PE consumer of d1b)
        br_ps = br_ps_t
        nc.tensor.matmul(br_ps, lhsT=scol, rhs=b2t, start=True, stop=True)
        br_sb = work.tile([1, C_OUT], BF16)
        nc.scalar.activation(br_sb, br_ps, mybir.ActivationFunctionType.Copy)

        # ---- G: 64 matmuls land [(bl,h), (i,p)] directly in two PSUM banks
        # bank A holds i 0:16, bank B i 16:32; each matmul writes a
        # contiguous [64, 32] block at partition offset 64*bl.
        g_psA = psum.tile([2 * H, 512], F32, name="g_psA")
        g_psB = psum.tile([2 * H, 512], F32, name="g_psB")
        for i in range(C_OUT):
            lhs = wexp_sb[:, i * 64:(i + 1) * 64]
            bank = g_psA if i < 16 else g_psB
            il = i % 16
            for bl, f2 in ((0, f2e), (1, f2o)):
                gout = bank[bl * H:(bl + 1) * H, il * NPAIR:(il + 1) * NPAIR]
                nc.tensor.matmul(gout, lhsT=lhs, rhs=f2, start=True, stop=True)
        g_sb = work.tile([2 * H, NPAIR * C_OUT], BF16)
        nc.scalar.activation(g_sb[:, 0:512], g_psA,
                             mybir.ActivationFunctionType.Copy)
        nc.scalar.activation(g_sb[:, 512:1024], g_psB,
                             mybir.ActivationFunctionType.Copy)

        # ---- T builds. Observer 1-element copies make each engine see the
        # cross-engine inputs once, so T ops need at most one sync wait.
        t_tiles = [None] * NPAIR
        pool_dummy = work.tile([1, 2], F32)
        nc.gpsimd.tensor_copy(pool_dummy[0:1, 0:1], ubT2[0:1, 0:1])
        nc.gpsimd.tensor_copy(pool_dummy[0:1, 1:2], negua2[0:1, 0:1])
        dve_obs = work.tile([1, 1], F32)
        nc.vector.tensor_copy(dve_obs, ubT2[0:1, 0:1])
        act_obs = work.tile([1, 1], BF16)
        nc.scalar.activation(act_obs, negua2[0:1, 0:1],
                             mybir.ActivationFunctionType.Copy)

        for p, eng in enumerate(sched):
            t_p = tpool.tile([2 * H, N], BF16, tag="T", name=f"t_{p}")
            t_tiles[p] = t_p
            if eng == "s":
                nc.scalar.activation(t_p, negua2,
                                     mybir.ActivationFunctionType.Relu,
                                     bias=ubT2[:, p:p + 1], scale=1.0)
            else:
                e = nc.vector if eng == "v" else nc.gpsimd
                e.tensor_scalar(out=t_p, in0=negua2,
                                scalar1=ubT2[:, p:p + 1], scalar2=0.0,
                                op0=mybir.AluOpType.add,
                                op1=mybir.AluOpType.max)

        # ---- T-gated PE warmers (keep p-state up while g copy is in flight)
        tw_ps = wk_ps[0:C_IN, 256:320]
        for w in range(N_TWARM):
            t_w = t_tiles[w]
            nc.tensor.matmul(tw_ps, lhsT=t_w[0:C_IN, 0:C_IN],
                             rhs=t_w[0:C_IN, 0:64], start=True, stop=True)

        # ---- accumulator: rank-1 b2 bias first, then the main chain
        acc = wk_ps[:, 384:384 + 2 * C_OUT]
        for ah in range(2):
            nc.tensor.matmul(acc[:, ah * C_OUT:(ah + 1) * C_OUT],
                             lhsT=ones_row, rhs=br_sb,
                             start=True, stop=False, skip_group_check=True)

        # PE observes the g copy once; main matmuls then only wait their T
        scrap = wk_ps[0:1, 320:321]
        nc.tensor.matmul(scrap, lhsT=g_sb[:, 0:1], rhs=g_sb[:, 0:1],
                         start=True, stop=True)

        for p in range(NPAIR):
            t_p = t_tiles[p]
            g_p = g_sb[:, p::NPAIR]
            for ah in range(2):
                nc.tensor.matmul(acc[:, ah * C_OUT:(ah + 1) * C_OUT],
                                 lhsT=t_p[:, ah * 128:(ah + 1) * 128],
                                 rhs=g_p,
                                 start=False, stop=(p == NPAIR - 1),
                                 skip_group_check=True)

        # ---- store
        out_sb = work.tile([128, 2 * C_OUT], BF16)
        nc.vector.tensor_copy(out_sb, acc)
        nc.sync.dma_start(out=outp, in_=out_sb)

    return nc


def shard_inputs(features, geometry, W1, b1, W2, b2) -> list[dict]:
    import ml_dtypes
    bf16 = ml_dtypes.bfloat16
    f = np.ascontiguousarray(np.asarray(features, np.float32))
    g = np.ascontiguousarray(np.asarray(geometry, np.float32))
    W1 = np.ascontiguousarray(np.asarray(W1, np.float32))
    b1 = np.ascontiguousarray(np.asarray(b1, np.float32))
    W2 = np.ascontiguousarray(np.asarray(W2, np.float32))
    b2 = np.ascontiguousarray(np.asarray(b2, np.float32))

    # wexp[j, i*64+h] = W2[h, i*32+j]
    w2r = W2.reshape(H, C_OUT, C_IN)            # [h, i, j]
    wexp = np.ascontiguousarray(
        w2r.transpose(2, 1, 0).reshape(C_IN, C_OUT * H)).astype(bf16)

    # w1bexp[(bl',x), (bl,h)] = delta(bl,bl') * [W1;b1][x, h]
    w1b = np.concatenate([W1, b1[None, :]], axis=0)      # [4, H]
    w1bexp = np.zeros((8, 128), np.float32)
    w1bexp[0:4, 0:64] = w1b
    w1bexp[4:8, 64:128] = w1b

    w1dup = np.concatenate([W1, W1], axis=1)             # [3, 128]

    b2t = np.ascontiguousarray(b2.reshape(C_OUT, C_IN).T)  # [j, i]

    maps = []
    for core in range(8):
        z, q = divmod(core, 4)
        sl = slice(q * BQ, (q + 1) * BQ)
        fq = f[z, sl]                                    # [64, j]
        gq = g[z, sl]                                    # [64, 3]

        d1 = np.zeros((D1_P, D1_W), np.float32)
        d1[0:3, 0:256] = g[z].T
        d1[0:3, 256:384] = w1dup
        # gb8[(bl'*4+x), p] = g[2p+bl', x] for x<3, 1.0 for x=3
        gb8 = np.zeros((8, NPAIR), np.float32)
        gb8[0:3, :] = gq[0::2].T
        gb8[3, :] = 1.0
        gb8[4:7, :] = gq[1::2].T
        gb8[7, :] = 1.0
        d1[0:8, 384:416] = gb8
        d1[0:8, 416:544] = w1bexp
        d1[:, 544:576] = fq[0::2].T                      # f2e [j, p]
        d1[:, 576:608] = fq[1::2].T                      # f2o
        d1[:, 608:672] = fq.T                            # fTb
        d1[:, 672:704] = b2t
        d1[0, 704:832] = 1.0

        maps.append({
            "d1": d1.astype(bf16),
            "wexp": wexp,
        })
    return maps


def unshard(parts: list[np.ndarray]) -> np.ndarray:
    out = np.zeros((Z, N, C_OUT), np.float32)
    for z in range(Z):
        for q in range(4):
            p = np.asarray(parts[4 * z + q], np.float32)   # [128, 64]
            out[z, 0:128] += p[:, 0:C_OUT]
            out[z, 128:256] += p[:, C_OUT:2 * C_OUT]
    return out


def kernel(**inputs) -> np.ndarray:
    nc = build_nc(debug=False)
    in_maps = shard_inputs(**inputs)
    res = run_bass_kernel_spmd(nc, in_maps, list(range(8)))
    return unshard([r["outp"] for r in res.results])


# revision 55
# speedup vs baseline: 1.1525x; 1.1525x over previous
"""Trainium2 Bass kernel for the pairwise-MLP geometric convolution.

Reference computes, per batch z:
    rel[a,b]   = g[b] - g[a]
    h[a,b,:]   = relu(rel @ W1 + b1)                      [N,N,H]
    k[a,b,:]   = h @ W2 + b2  -> [N,N,C_OUT,C_IN]
    out[a,i]   = sum_{b,j} k[a,b,i,j] * f[b,j]

Factorization (k never materialized):
    U = g @ W1
    T[(b,h), a] = relu(U[b,h] + b1[h] - U[a,h])
    G[(b,h), i] = sum_j W2[h, i*C_IN+j] * f[b,j]
    out[a, i]   = sum_{(b,h)} T[(b,h),a] * G[(b,h),i]
                + sum_j b2[i,j] * (sum_b f[b,j])

Sharding over 8 cores: z (2) x b-quarter (4); each core computes the full
[a=256, i=32] partial for its 64 b's; host sums quarters.

Per-core dataflow (all matmul operands bf16, PSUM accumulation fp32):
  * K-chunks of 128 = (b-pair bl in {0,1}) x (h=64). 32 pairs.
  * G lands DIRECTLY in [(bl,h), (i,p)] PSUM layout via 64 small matmuls
    (lhsT = W2 packed [j, h]-per-i on host, rhs = even/odd-b features,
    out partition-offset 64*bl) -- no reshape DMAs, one ACT copy per bank.
  * Ub+b1 lands directly as [(bl,h), p] via one delta-expanded matmul
    (lhsT rows (bl',x) = [W1;b1] on the bl=bl' diagonal, rhs = paired g).
  * T tiles [128, 256] built by DVE (bf16 packed tensor_scalar, ~127ns),
    ACT (activation relu with per-partition bias), and GPSIMD, split
    24/2/6 to finish together; the last pair's two a-halves are built on
    DVE and GPSIMD in parallel to halve the closing tile's latency.
  * Main chain: out[a_half, i] += t_p[:, half].T @ g_p  -- 32-row matmuls,
    T stationary, so the PE streams 2048 rows instead of 8192.
  * b2 bias via rank-1 matmuls (ones x (b2^T @ sum_b f)).

Critical-path measures (validated against the TimelineSim cost model):
  * The three input-load DMAs are relocated post-scheduling to the very
    top of the pre-barrier SP preamble stream (_hoist_input_dmas), ahead
    of SP's monotonic-sem RegisterMoves (pushed behind the barrier join),
    so their HWDGE prep + DGE latency fully overlaps the opening barrier.
  * The Bass-preamble const memsets are rerouted off the slow GPSIMD queue.
  * PSUM tiles are never shared between independent consumers (the Tile
    dep tracker chains all accessors of a PSUM tile, serializing readers).
  * One 128-column PE warmer raises the p-state before the U matmuls.

Hardware constraint honored throughout: the walrus codegen accepts at most
ONE sync-wait per TPB instruction; consumers observe multi-engine deps
through single-wait observer ops (Pool/DVE/ACT 1-element copies, PE scrap
matmul).
"""

import sys

import numpy as np

_TRN_REPO = "/opt/trn_rl_repo"
if _TRN_REPO not in sys.path:
    sys.path.insert(0, _TRN_REPO)

from contextlib import ExitStack

import concourse.bass as bass
import concourse.mybir as mybir
import concourse.tile as tile
from concourse.bass_utils import run_bass_kernel_spmd

from concourse.vector_clock import ScopedClock

# The walrus codegen used on the axon/PJRT path accepts at most ONE sync-wait
# per TPB instruction. Tile's kernel-tail drain aggregates a wait for every
# live semaphore onto a single Drain, which walrus rejects. Patch the tail to
# spread those waits across single-wait SP nops before an unadorned drain.
_orig_drain_and_barrier = tile.TileContext._drain_and_barrier


def _split_wait_drain_and_barrier(self, tick_clock, wait_clock):
    nc = self.nc
    probe = nc.sync.nop(nofuse=True)
    wait_clock.add_sem_waits(probe.ins, ScopedClock({None: tick_clock.global_clock}))
    si = probe.ins.sync_info
    waits = list(si.on_wait) if si is not None and si.on_wait else []
    if len(waits) > 1:
        probe.ins.sync_info = mybir.SyncInfo(on_wait=waits[:1], on_update=[])
        for w in waits[1:]:
            extra = nc.sync.nop(nofuse=True)
            extra.ins.sync_info = mybir.SyncInfo(on_wait=[w], on_update=[])
    nc.sync.drain()
    nc.all_engine_barrier()
    popped = nc._tile_sem_poison_stack.pop()
    assert popped is self._sem_poison
    nc.clear_and_free_semaphores(list(self.sems.allocated().values()))


tile.TileContext._drain_and_barrier = _split_wait_drain_and_barrier

# The Bass constructor registers four const APs via gpsimd.memset; Pool's Q7
# launch overhead puts ~0.4us of serial work ahead of the opening all-engine
# barrier. Reroute those preamble memsets to the (faster, otherwise idle) DVE
# queue. The barrier after them still guarantees completion.
_orig_bass_init = bass.Bass.__init__


def _patched_bass_init(self, *a, **k):
    self._in_preamble_init = True
    try:
        _orig_bass_init(self, *a, **k)
    finally:
        self._in_preamble_init = False


_orig_memset = bass.BassEitherVectorEngine.memset


def _patched_memset(self, ap, constant):
    b = getattr(self, "bass", None)
    if b is not None and getattr(b, "_in_preamble_init", False):
        if constant == 0.0 and ap.dtype == mybir.dt.float32:
            return b.scalar.memzero(ap)
        return _orig_memset(b.vector, ap, constant)
    return _orig_memset(self, ap, constant)


bass.Bass.__init__ = _patched_bass_init
bass.BassEitherVectorEngine.memset = _patched_memset

F32 = mybir.dt.float32
BF16 = mybir.dt.bfloat16
Z, N, C_IN, C_OUT, H = 2, 256, 32, 32, 64
BQ = 64          # b-points per core (N / 4 quarters)
NPAIR = BQ // 2  # 32 K-chunks of (2 b x 64 h) = 128

# d1 [32, 832]: rows 0:3 -> gT (cols 0:256), W1dup (256:384);
#               rows 0:8 -> gb8 (384:416), w1bexp (416:544);
#               rows 0:32 -> f2e (544:576), f2o (576:608), fTb (608:672),
#               b2t (672:704), ones row 0 (704:832)
D1_P, D1_W = 32, 832
# wexp [32, 2048]: [j, i*64+h] = W2[h, i*32+j]
WEXP_P, WEXP_W = 32, 2048

# T-build schedule: engine of each build slot in production order.
# v=DVE (127ns), s=ACT (398ns), g=GPSIMD (451ns). ACT also does the
# ubT2 copy first and the G PSUM->SBUF copy mid-stream.
N_V, N_S, N_G = 24, 2, 6
N_WARM = 1        # single p-state warmer ahead of the U matmuls


def _t_schedule():
    """Interleave T-builds across engines by estimated completion time.

    Returns a list of engine codes, one per pair, in estimated completion
    order (= PE consumption order).
    """
    # times are estimated SEMAPHORE-visible completion (engine time plus
    # ack/prop latency), so the PE consumption order matches readiness:
    # one ACT tile hides in the pre-gcopy gap, the rest follow the copies.
    slots = []
    for k in range(N_V):
        slots.append((3450 + 127 * (k + 1), "v"))
    for k in range(N_S):
        slots.append((5180 + 398 * (k + 1), "s"))
    for k in range(N_G):
        slots.append((3630 + 451 * (k + 1), "g"))
    slots.sort()
    return [e for _, e in slots]


def build_nc(debug: bool = False) -> bass.Bass:
    nc = bass.Bass("TRN2", target_bir_lowering=False, debug=debug, num_devices=8)

    d1 = nc.dram_tensor("d1", [D1_P, D1_W], BF16, kind="ExternalInput").ap()
    wexp = nc.dram_tensor("wexp", [WEXP_P, WEXP_W], BF16, kind="ExternalInput").ap()
    outp = nc.dram_tensor("outp", [128, 2 * C_OUT], BF16, kind="ExternalOutput").ap()

    sched = _t_schedule()

    with tile.TileContext(nc) as tc, ExitStack() as ctx:
        consts = ctx.enter_context(tc.tile_pool(name="consts", bufs=1))
        work = ctx.enter_context(tc.tile_pool(name="work", bufs=1))
        # every T tile gets its own slot so no T-op waits on a PE release
        tpool = ctx.enter_context(tc.tile_pool(name="tpool", bufs=NPAIR))
        psum = ctx.enter_context(tc.tile_pool(name="psum", bufs=1, space="PSUM"))

        # ---- input loads: both on the SP queue, d1 first
        d1_sb = consts.tile([D1_P, D1_W], BF16)
        nc.sync.dma_start(out=d1_sb, in_=d1)
        wexp_sb = consts.tile([WEXP_P, WEXP_W], BF16)
        nc.sync.dma_start(out=wexp_sb[:, 0:1024], in_=wexp[:, 0:1024])
        nc.sync.dma_start(out=wexp_sb[:, 1024:2048], in_=wexp[:, 1024:2048])

        gT = d1_sb[0:3, 0:256]
        w1dup = d1_sb[0:3, 256:384]
        gb8 = d1_sb[0:8, 384:416]
        w1bexp = d1_sb[0:8, 416:544]
        f2e = d1_sb[0:32, 544:576]
        f2o = d1_sb[0:32, 576:608]
        fTb = d1_sb[0:32, 608:672]
        b2t = d1_sb[0:32, 672:704]
        ones_row = d1_sb[0:1, 704:832]

        # ---- PE p-state warm-up: memset a row on DVE, then dummy matmuls
        wsrc = work.tile([1, 256], BF16)
        nc.vector.memset(wsrc, 1.0)
        # PSUM tiles are access-chained by the dep tracker: give every
        # independently-consumed producer its own tile.
        wk_ps = psum.tile([128, 512], F32, name="wk_ps")
        u_ps = psum.tile([128, 256], F32, name="u_ps")
        ubbr_ps = psum.tile([128, 64], F32, name="ubbr_ps")
        br_ps_t = psum.tile([1, C_OUT], F32, name="br_ps_t")
        warm_ps = wk_ps[0:1, 0:128]
        for _ in range(N_WARM):
            nc.tensor.matmul(warm_ps, lhsT=wsrc[0:1, 0:1], rhs=wsrc[0:1, 0:128],
                             start=True, stop=True)

        # ---- U matmuls (dup'd over both partition halves); ub first so the
        # ACT ubT2 copy (whose ack gates the first DVE T) starts earliest
        ub_ps = ubbr_ps[:, 0:NPAIR]
        nc.tensor.matmul(ub_ps, lhsT=w1bexp, rhs=gb8, start=True, stop=True)
        uaT2_ps = u_ps
        nc.tensor.matmul(uaT2_ps, lhsT=w1dup, rhs=gT, start=True, stop=True)

        # ---- ubT2 (Ub + b1 in [(bl,h), p] layout) to SBUF on ACT
        ubT2 = work.tile([2 * H, NPAIR], F32)
        nc.scalar.activation(ubT2, ub_ps, mybir.ActivationFunctionType.Copy)

        # ---- small DVE chain: scol reduce, then negua2, then birow copy
        scol = work.tile([C_IN, 1], BF16)
        with nc.allow_low_precision(reason="bf16 matmul operand; one rounding"):
            nc.vector.tensor_reduce(out=scol, in_=fTb,
                                    axis=mybir.AxisListType.X,
                                    op=mybir.AluOpType.add)
        negua2 = work.tile([2 * H, N], BF16)
        nc.vector.tensor_scalar(out=negua2, in0=uaT2_ps, scalar1=-1.0,
                                scalar2=None, op0=mybir.AluOpType.mult)

        # b2 bias row: br = scol^T-contracted b2t  (biasrow mm waits DVE>=scol,
        # which also subsumes the d1 DMA for every later PE consumer of d1)
        br_ps = br_ps_t
        nc.tensor.matmul(br_ps, lhsT=scol, rhs=b2t, start=True, stop=True)
        br_sb = work.tile([1, C_OUT], BF16)
        nc.scalar.activation(br_sb, br_ps, mybir.ActivationFunctionType.Copy)

        # ---- G: 64 matmuls land [(bl,h), (i,p)] directly in two PSUM banks
        # bank A holds i 0:16, bank B i 16:32; each matmul writes a
        # contiguous [64, 32] block at partition offset 64*bl.
        g_psA = psum.tile([2 * H, 512], F32, name="g_psA")
        g_psB = psum.tile([2 * H, 512], F32, name="g_psB")
        for i in range(C_OUT):
            lhs = wexp_sb[:, i * 64:(i + 1) * 64]
            bank = g_psA if i < 16 else g_psB
            il = i % 16
            for bl, f2 in ((0, f2e), (1, f2o)):
                gout = bank[bl * H:(bl + 1) * H, il * NPAIR:(il + 1) * NPAIR]
                nc.tensor.matmul(gout, lhsT=lhs, rhs=f2, start=True, stop=True)
        g_sb = work.tile([2 * H, NPAIR * C_OUT], BF16)
        nc.scalar.activation(g_sb[:, 0:512], g_psA,
                             mybir.ActivationFunctionType.Copy)
        nc.scalar.activation(g_sb[:, 512:1024], g_psB,
                             mybir.ActivationFunctionType.Copy)

        # ---- T builds. Observer 1-element copies make each engine see the
        # cross-engine inputs once, so T ops need at most one sync wait.
        t_tiles = [None] * NPAIR
        pool_dummy = work.tile([1, 2], F32)
        nc.gpsimd.tensor_copy(pool_dummy[0:1, 0:1], ubT2[0:1, 0:1])
        dve_obs = work.tile([1, 1], F32)
        nc.vector.tensor_copy(dve_obs, ubT2[0:1, 0:1])
        act_obs = work.tile([1, 1], BF16)
        nc.scalar.activation(act_obs, negua2[0:1, 0:1],
                             mybir.ActivationFunctionType.Copy)

        for p, eng in enumerate(sched):
            t_p = tpool.tile([2 * H, N], BF16, tag="T", name=f"t_{p}")
            t_tiles[p] = t_p
            if p == NPAIR - 1:
                # last pair: build the two a-halves on DVE and GPSIMD in
                # parallel so the closing tile's latency chain is halved
                nc.vector.tensor_scalar(out=t_p[:, 0:128],
                                        in0=negua2[:, 0:128],
                                        scalar1=ubT2[:, p:p + 1], scalar2=0.0,
                                        op0=mybir.AluOpType.add,
                                        op1=mybir.AluOpType.max)
                nc.scalar.activation(t_p[:, 128:256], negua2[:, 128:256],
                                     mybir.ActivationFunctionType.Relu,
                                     bias=ubT2[:, p:p + 1], scale=1.0)
            elif eng == "s":
                nc.scalar.activation(t_p, negua2,
                                     mybir.ActivationFunctionType.Relu,
                                     bias=ubT2[:, p:p + 1], scale=1.0)
            else:
                e = nc.vector if eng == "v" else nc.gpsimd
                e.tensor_scalar(out=t_p, in0=negua2,
                                scalar1=ubT2[:, p:p + 1], scalar2=0.0,
                                op0=mybir.AluOpType.add,
                                op1=mybir.AluOpType.max)

        # ---- accumulator: rank-1 b2 bias first, then the main chain
        acc = wk_ps[:, 384:384 + 2 * C_OUT]
        for ah in range(2):
            nc.tensor.matmul(acc[:, ah * C_OUT:(ah + 1) * C_OUT],
                             lhsT=ones_row, rhs=br_sb,
                             start=True, stop=False, skip_group_check=True)

        # PE observes the g copy once; main matmuls then only wait their T
        scrap = wk_ps[0:1, 320:321]
        nc.tensor.matmul(scrap, lhsT=g_sb[:, 0:1], rhs=g_sb[:, 0:1],
                         start=True, stop=True)

        for p in range(NPAIR):
            t_p = t_tiles[p]
            g_p = g_sb[:, p::NPAIR]
            for ah in range(2):
                nc.tensor.matmul(acc[:, ah * C_OUT:(ah + 1) * C_OUT],
                                 lhsT=t_p[:, ah * 128:(ah + 1) * 128],
                                 rhs=g_p,
                                 start=False, stop=(p == NPAIR - 1),
                                 skip_group_check=True)

        # ---- store
        out_sb = work.tile([128, 2 * C_OUT], BF16)
        nc.vector.tensor_copy(out_sb, acc)
        nc.sync.dma_start(out=outp, in_=out_sb)

    _hoist_input_dmas(nc)
    return nc


def _hoist_input_dmas(nc: bass.Bass) -> None:
    """Move the three input-load DMACopys into the preamble block, ahead of
    the opening all-engine barrier. Their HWDGE prep + DGE latency then
    overlaps the barrier, so the first payload lands ~0.65us earlier.
    Consumers' semaphore waits are untouched -- the sems just fire earlier."""
    fn = nc.m.functions[0]
    blocks = fn.blocks
    pre = blocks[0]

    moved = []          # the three SP input-load DMACopys
    moved_ms = []       # the warm-up source memset (DVE)
    for blk in blocks[1:]:
        insns = blk.instructions
        keep = []
        for ins in insns:
            si = ins.sync_info
            has_wait = bool(si and si.on_wait)
            if (len(moved) < 3 and ins.opcode == "DMACopy"
                    and str(ins.engine) == "EngineType.SP" and not has_wait):
                moved.append(ins)
            elif (not moved_ms and ins.opcode == "Memset"
                    and str(ins.engine) == "EngineType.DVE"):
                moved_ms.append(ins)
            else:
                keep.append(ins)
        if len(keep) != len(insns):
            blk.instructions = keep
        if len(moved) == 3 and moved_ms:
            break
    assert len(moved) == 3, f"expected 3 input DMAs, found {len(moved)}"
    assert len(moved_ms) == 1, "expected the wsrc memset"

    # insert the DMAs at the very top of the SP stream, and push SP's
    # preamble RegisterMoves (they only zero monotonic-sem scratch
    # registers, unused by this kernel) behind the barrier join so they
    # don't delay it
    pre_insns = pre.instructions
    out = []
    sp_regmoves = []
    placed_dma = placed_ms = False
    for ins in pre_insns:
        if not placed_dma and str(ins.engine) == "EngineType.SP":
            out.extend(moved)
            placed_dma = True
        if (ins.opcode == "RegisterMove"
                and str(ins.engine) == "EngineType.SP"):
            sp_regmoves.append(ins)
            continue
        if (not placed_ms and ins.opcode == "Drain"
                and str(ins.engine) == "EngineType.DVE"):
            out.extend(moved_ms)
            placed_ms = True
        out.append(ins)
        if (ins.opcode == "EventSemaphore" and sp_regmoves
                and str(ins.engine) == "EngineType.SP"):
            out.extend(sp_regmoves)
            sp_regmoves = []
    assert placed_dma and placed_ms and not sp_regmoves
    pre.instructions = out


def shard_inputs(features, geometry, W1, b1, W2, b2) -> list[dict]:
    import ml_dtypes
    bf16 = ml_dtypes.bfloat16
    f = np.ascontiguousarray(np.asarray(features, np.float32))
    g = np.ascontiguousarray(np.asarray(geometry, np.float32))
    W1 = np.ascontiguousarray(np.asarray(W1, np.float32))
    b1 = np.ascontiguousarray(np.asarray(b1, np.float32))
    W2 = np.ascontiguousarray(np.asarray(W2, np.float32))
    b2 = np.ascontiguousarray(np.asarray(b2, np.float32))

    # wexp[j, i*64+h] = W2[h, i*32+j]
    w2r = W2.reshape(H, C_OUT, C_IN)            # [h, i, j]
    wexp = np.ascontiguousarray(
        w2r.transpose(2, 1, 0).reshape(C_IN, C_OUT * H)).astype(bf16)

    # w1bexp[(bl',x), (bl,h)] = delta(bl,bl') * [W1;b1][x, h]
    w1b = np.concatenate([W1, b1[None, :]], axis=0)      # [4, H]
    w1bexp = np.zeros((8, 128), np.float32)
    w1bexp[0:4, 0:64] = w1b
    w1bexp[4:8, 64:128] = w1b

    w1dup = np.concatenate([W1, W1], axis=1)             # [3, 128]

    b2t = np.ascontiguousarray(b2.reshape(C_OUT, C_IN).T)  # [j, i]

    maps = []
    for core in range(8):
        z, q = divmod(core, 4)
        sl = slice(q * BQ, (q + 1) * BQ)
        fq = f[z, sl]                                    # [64, j]
        gq = g[z, sl]                                    # [64, 3]

        d1 = np.zeros((D1_P, D1_W), np.float32)
        d1[0:3, 0:256] = g[z].T
        d1[0:3, 256:384] = w1dup
        # gb8[(bl'*4+x), p] = g[2p+bl', x] for x<3, 1.0 for x=3
        gb8 = np.zeros((8, NPAIR), np.float32)
        gb8[0:3, :] = gq[0::2].T
        gb8[3, :] = 1.0
        gb8[4:7, :] = gq[1::2].T
        gb8[7, :] = 1.0
        d1[0:8, 384:416] = gb8
        d1[0:8, 416:544] = w1bexp
        d1[:, 544:576] = fq[0::2].T                      # f2e [j, p]
        d1[:, 576:608] = fq[1::2].T                      # f2o
        d1[:, 608:672] = fq.T                            # fTb
        d1[:, 672:704] = b2t
        d1[0, 704:832] = 1.0

        maps.append({
            "d1": d1.astype(bf16),
            "wexp": wexp,
        })
    return maps


def unshard(parts: list[np.ndarray]) -> np.ndarray:
    out = np.zeros((Z, N, C_OUT), np.float32)
    for z in range(Z):
        for q in range(4):
            p = np.asarray(parts[4 * z + q], np.float32)   # [128, 64]
            out[z, 0:128] += p[:, 0:C_OUT]
            out[z, 128:256] += p[:, C_OUT:2 * C_OUT]
    return out


def kernel(**inputs) -> np.ndarray:
    nc = build_nc(debug=False)
    in_maps = shard_inputs(**inputs)
    res = run_bass_kernel_spmd(nc, in_maps, list(range(8)))
    return unshard([r["outp"] for r in res.results])


# revision 56
# speedup vs baseline: 1.1548x; 1.0020x over previous
"""Trainium2 Bass kernel for the pairwise-MLP geometric convolution.

Reference computes, per batch z:
    rel[a,b]   = g[b] - g[a]
    h[a,b,:]   = relu(rel @ W1 + b1)                      [N,N,H]
    k[a,b,:]   = h @ W2 + b2  -> [N,N,C_OUT,C_IN]
    out[a,i]   = sum_{b,j} k[a,b,i,j] * f[b,j]

Factorization (k never materialized):
    U = g @ W1
    T[(b,h), a] = relu(U[b,h] + b1[h] - U[a,h])
    G[(b,h), i] = sum_j W2[h, i*C_IN+j] * f[b,j]
    out[a, i]   = sum_{(b,h)} T[(b,h),a] * G[(b,h),i]
                + sum_j b2[i,j] * (sum_b f[b,j])

Sharding over 8 cores: z (2) x b-quarter (4); each core computes the full
[a=256, i=32] partial for its 64 b's; host sums quarters.

Per-core dataflow (all matmul operands bf16, PSUM accumulation fp32):
  * K-chunks of 128 = (b-pair bl in {0,1}) x (h=64). 32 pairs.
  * G lands DIRECTLY in [(bl,h), (i,p)] PSUM layout via 64 small matmuls
    (lhsT = W2 packed [j, h]-per-i on host, rhs = even/odd-b features,
    out partition-offset 64*bl) -- no reshape DMAs, one ACT copy per bank.
  * Ub+b1 lands directly as [(bl,h), p] via one delta-expanded matmul
    (lhsT rows (bl',x) = [W1;b1] on the bl=bl' diagonal, rhs = paired g).
  * T tiles [128, 256] built by DVE (bf16 packed tensor_scalar, ~127ns),
    ACT (activation relu with per-partition bias), and GPSIMD, split
    24/2/6 to finish together; the last pair's two a-halves are built on
    DVE and GPSIMD in parallel to halve the closing tile's latency.
  * Main chain: out[a_half, i] += t_p[:, half].T @ g_p  -- 32-row matmuls,
    T stationary, so the PE streams 2048 rows instead of 8192.
  * b2 bias via rank-1 matmuls (ones x (b2^T @ sum_b f)).

Critical-path measures (validated against the TimelineSim cost model):
  * The three input-load DMAs are relocated post-scheduling to the very
    top of the pre-barrier SP preamble stream (_hoist_input_dmas), ahead
    of SP's monotonic-sem RegisterMoves (pushed behind the barrier join),
    so their HWDGE prep + DGE latency fully overlaps the opening barrier.
  * The Bass-preamble const memsets are rerouted off the slow GPSIMD queue.
  * PSUM tiles are never shared between independent consumers (the Tile
    dep tracker chains all accessors of a PSUM tile, serializing readers).
  * One 128-column PE warmer raises the p-state before the U matmuls.

Hardware constraint honored throughout: the walrus codegen accepts at most
ONE sync-wait per TPB instruction; consumers observe multi-engine deps
through single-wait observer ops (Pool/DVE/ACT 1-element copies, PE scrap
matmul).
"""

import sys

import numpy as np

_TRN_REPO = "/opt/trn_rl_repo"
if _TRN_REPO not in sys.path:
    sys.path.insert(0, _TRN_REPO)

from contextlib import ExitStack

import concourse.bass as bass
import concourse.mybir as mybir
import concourse.tile as tile
from concourse.bass_utils import run_bass_kernel_spmd

from concourse.vector_clock import ScopedClock

# The walrus codegen used on the axon/PJRT path accepts at most ONE sync-wait
# per TPB instruction. Tile's kernel-tail drain aggregates a wait for every
# live semaphore onto a single Drain, which walrus rejects. Patch the tail to
# spread those waits across single-wait SP nops before an unadorned drain.
_orig_drain_and_barrier = tile.TileContext._drain_and_barrier


def _split_wait_drain_and_barrier(self, tick_clock, wait_clock):
    nc = self.nc
    probe = nc.sync.nop(nofuse=True)
    wait_clock.add_sem_waits(probe.ins, ScopedClock({None: tick_clock.global_clock}))
    si = probe.ins.sync_info
    waits = list(si.on_wait) if si is not None and si.on_wait else []
    if len(waits) > 1:
        probe.ins.sync_info = mybir.SyncInfo(on_wait=waits[:1], on_update=[])
        for w in waits[1:]:
            extra = nc.sync.nop(nofuse=True)
            extra.ins.sync_info = mybir.SyncInfo(on_wait=[w], on_update=[])
    nc.sync.drain()
    nc.all_engine_barrier()
    popped = nc._tile_sem_poison_stack.pop()
    assert popped is self._sem_poison
    nc.clear_and_free_semaphores(list(self.sems.allocated().values()))


tile.TileContext._drain_and_barrier = _split_wait_drain_and_barrier

# The Bass constructor registers four const APs via gpsimd.memset; Pool's Q7
# launch overhead puts ~0.4us of serial work ahead of the opening all-engine
# barrier. Reroute those preamble memsets to the (faster, otherwise idle) DVE
# queue. The barrier after them still guarantees completion.
_orig_bass_init = bass.Bass.__init__


def _patched_bass_init(self, *a, **k):
    self._in_preamble_init = True
    try:
        _orig_bass_init(self, *a, **k)
    finally:
        self._in_preamble_init = False


_orig_memset = bass.BassEitherVectorEngine.memset


def _patched_memset(self, ap, constant):
    b = getattr(self, "bass", None)
    if b is not None and getattr(b, "_in_preamble_init", False):
        if constant == 0.0 and ap.dtype == mybir.dt.float32:
            return b.scalar.memzero(ap)
        return _orig_memset(b.vector, ap, constant)
    return _orig_memset(self, ap, constant)


bass.Bass.__init__ = _patched_bass_init
bass.BassEitherVectorEngine.memset = _patched_memset

F32 = mybir.dt.float32
BF16 = mybir.dt.bfloat16
Z, N, C_IN, C_OUT, H = 2, 256, 32, 32, 64
BQ = 64          # b-points per core (N / 4 quarters)
NPAIR = BQ // 2  # 32 K-chunks of (2 b x 64 h) = 128

# d1 [32, 704]: rows 0:3 -> gT (cols 0:256), W1dup (256:384);
#               rows 0:8 -> gb8 (384:416), w1bexp (416:544);
#               rows 0:32 -> f2e (544:576), f2o (576:608), fTb (608:672),
#               b2t (672:704). The rank-1-bias ones row reuses wsrc.
D1_P, D1_W = 32, 704
# wexp [32, 2048]: [j, i*64+h] = W2[h, i*32+j]
WEXP_P, WEXP_W = 32, 2048

# T-build schedule: engine of each build slot in production order.
# v=DVE (127ns), s=ACT (398ns), g=GPSIMD (451ns). ACT also does the
# ubT2 copy first and the G PSUM->SBUF copy mid-stream.
N_V, N_S, N_G = 24, 2, 6
N_WARM = 1        # single p-state warmer ahead of the U matmuls


def _t_schedule():
    """Interleave T-builds across engines by estimated completion time.

    Returns a list of engine codes, one per pair, in estimated completion
    order (= PE consumption order).
    """
    # times are estimated SEMAPHORE-visible completion (engine time plus
    # ack/prop latency), so the PE consumption order matches readiness:
    # one ACT tile hides in the pre-gcopy gap, the rest follow the copies.
    slots = []
    for k in range(N_V):
        slots.append((3450 + 127 * (k + 1), "v"))
    for k in range(N_S):
        slots.append((5180 + 398 * (k + 1), "s"))
    for k in range(N_G):
        slots.append((3630 + 451 * (k + 1), "g"))
    slots.sort()
    return [e for _, e in slots]


def build_nc(debug: bool = False) -> bass.Bass:
    nc = bass.Bass("TRN2", target_bir_lowering=False, debug=debug, num_devices=8)

    d1 = nc.dram_tensor("d1", [D1_P, D1_W], BF16, kind="ExternalInput").ap()
    wexp = nc.dram_tensor("wexp", [WEXP_P, WEXP_W], BF16, kind="ExternalInput").ap()
    outp = nc.dram_tensor("outp", [128, 2 * C_OUT], BF16, kind="ExternalOutput").ap()

    sched = _t_schedule()

    with tile.TileContext(nc) as tc, ExitStack() as ctx:
        consts = ctx.enter_context(tc.tile_pool(name="consts", bufs=1))
        work = ctx.enter_context(tc.tile_pool(name="work", bufs=1))
        # every T tile gets its own slot so no T-op waits on a PE release
        tpool = ctx.enter_context(tc.tile_pool(name="tpool", bufs=NPAIR))
        psum = ctx.enter_context(tc.tile_pool(name="psum", bufs=1, space="PSUM"))

        # ---- input loads: both on the SP queue, d1 first
        d1_sb = consts.tile([D1_P, D1_W], BF16)
        nc.sync.dma_start(out=d1_sb, in_=d1)
        wexp_sb = consts.tile([WEXP_P, WEXP_W], BF16)
        nc.sync.dma_start(out=wexp_sb[:, 0:1024], in_=wexp[:, 0:1024])
        nc.sync.dma_start(out=wexp_sb[:, 1024:2048], in_=wexp[:, 1024:2048])

        gT = d1_sb[0:3, 0:256]
        w1dup = d1_sb[0:3, 256:384]
        gb8 = d1_sb[0:8, 384:416]
        w1bexp = d1_sb[0:8, 416:544]
        f2e = d1_sb[0:32, 544:576]
        f2o = d1_sb[0:32, 576:608]
        fTb = d1_sb[0:32, 608:672]
        b2t = d1_sb[0:32, 672:704]


        # ---- PE p-state warm-up: memset a row on DVE, then dummy matmuls
        wsrc = work.tile([1, 256], BF16)
        nc.vector.memset(wsrc, 1.0)
        ones_row = wsrc[0:1, 0:128]
        # PSUM tiles are access-chained by the dep tracker: give every
        # independently-consumed producer its own tile.
        wk_ps = psum.tile([128, 512], F32, name="wk_ps")
        u_ps = psum.tile([128, 256], F32, name="u_ps")
        ubbr_ps = psum.tile([128, 64], F32, name="ubbr_ps")
        br_ps_t = psum.tile([1, C_OUT], F32, name="br_ps_t")
        warm_ps = wk_ps[0:1, 0:128]
        for _ in range(N_WARM):
            nc.tensor.matmul(warm_ps, lhsT=wsrc[0:1, 0:1], rhs=wsrc[0:1, 0:128],
                             start=True, stop=True)

        # ---- U matmuls (dup'd over both partition halves); ub first so the
        # ACT ubT2 copy (whose ack gates the first DVE T) starts earliest
        ub_ps = ubbr_ps[:, 0:NPAIR]
        nc.tensor.matmul(ub_ps, lhsT=w1bexp, rhs=gb8, start=True, stop=True)
        uaT2_ps = u_ps
        nc.tensor.matmul(uaT2_ps, lhsT=w1dup, rhs=gT, start=True, stop=True)

        # ---- ubT2 (Ub + b1 in [(bl,h), p] layout) to SBUF on ACT
        ubT2 = work.tile([2 * H, NPAIR], F32)
        nc.scalar.activation(ubT2, ub_ps, mybir.ActivationFunctionType.Copy)

        # ---- small DVE chain: scol reduce, then negua2, then birow copy
        scol = work.tile([C_IN, 1], BF16)
        with nc.allow_low_precision(reason="bf16 matmul operand; one rounding"):
            nc.vector.tensor_reduce(out=scol, in_=fTb,
                                    axis=mybir.AxisListType.X,
                                    op=mybir.AluOpType.add)
        negua2 = work.tile([2 * H, N], BF16)
        nc.vector.tensor_scalar(out=negua2, in0=uaT2_ps, scalar1=-1.0,
                                scalar2=None, op0=mybir.AluOpType.mult)

        # b2 bias row: br = scol^T-contracted b2t  (biasrow mm waits DVE>=scol,
        # which also subsumes the d1 DMA for every later PE consumer of d1)
        br_ps = br_ps_t
        nc.tensor.matmul(br_ps, lhsT=scol, rhs=b2t, start=True, stop=True)
        br_sb = work.tile([1, C_OUT], BF16)
        nc.scalar.activation(br_sb, br_ps, mybir.ActivationFunctionType.Copy)

        # ---- G: 64 matmuls land [(bl,h), (i,p)] directly in two PSUM banks
        # bank A holds i 0:16, bank B i 16:32; each matmul writes a
        # contiguous [64, 32] block at partition offset 64*bl.
        g_psA = psum.tile([2 * H, 512], F32, name="g_psA")
        g_psB = psum.tile([2 * H, 512], F32, name="g_psB")
        for i in range(C_OUT):
            lhs = wexp_sb[:, i * 64:(i + 1) * 64]
            bank = g_psA if i < 16 else g_psB
            il = i % 16
            for bl, f2 in ((0, f2e), (1, f2o)):
                gout = bank[bl * H:(bl + 1) * H, il * NPAIR:(il + 1) * NPAIR]
                nc.tensor.matmul(gout, lhsT=lhs, rhs=f2, start=True, stop=True)
        g_sb = work.tile([2 * H, NPAIR * C_OUT], BF16)
        nc.scalar.activation(g_sb[:, 0:512], g_psA,
                             mybir.ActivationFunctionType.Copy)
        nc.scalar.activation(g_sb[:, 512:1024], g_psB,
                             mybir.ActivationFunctionType.Copy)

        # ---- T builds. Observer 1-element copies make each engine see the
        # cross-engine inputs once, so T ops need at most one sync wait.
        t_tiles = [None] * NPAIR
        pool_dummy = work.tile([1, 2], F32)
        nc.gpsimd.tensor_copy(pool_dummy[0:1, 0:1], ubT2[0:1, 0:1])
        dve_obs = work.tile([1, 1], F32)
        nc.vector.tensor_copy(dve_obs, ubT2[0:1, 0:1])
        act_obs = work.tile([1, 1], BF16)
        nc.scalar.activation(act_obs, negua2[0:1, 0:1],
                             mybir.ActivationFunctionType.Copy)

        for p, eng in enumerate(sched):
            t_p = tpool.tile([2 * H, N], BF16, tag="T", name=f"t_{p}")
            t_tiles[p] = t_p
            if p == NPAIR - 1:
                # last pair: build the two a-halves on DVE and GPSIMD in
                # parallel so the closing tile's latency chain is halved
                nc.vector.tensor_scalar(out=t_p[:, 0:128],
                                        in0=negua2[:, 0:128],
                                        scalar1=ubT2[:, p:p + 1], scalar2=0.0,
                                        op0=mybir.AluOpType.add,
                                        op1=mybir.AluOpType.max)
                nc.scalar.activation(t_p[:, 128:256], negua2[:, 128:256],
                                     mybir.ActivationFunctionType.Relu,
                                     bias=ubT2[:, p:p + 1], scale=1.0)
            elif eng == "s":
                nc.scalar.activation(t_p, negua2,
                                     mybir.ActivationFunctionType.Relu,
                                     bias=ubT2[:, p:p + 1], scale=1.0)
            else:
                e = nc.vector if eng == "v" else nc.gpsimd
                e.tensor_scalar(out=t_p, in0=negua2,
                                scalar1=ubT2[:, p:p + 1], scalar2=0.0,
                                op0=mybir.AluOpType.add,
                                op1=mybir.AluOpType.max)

        # ---- accumulator: rank-1 b2 bias first, then the main chain
        acc = wk_ps[:, 384:384 + 2 * C_OUT]
        for ah in range(2):
            nc.tensor.matmul(acc[:, ah * C_OUT:(ah + 1) * C_OUT],
                             lhsT=ones_row, rhs=br_sb,
                             start=True, stop=False, skip_group_check=True)

        # PE observes the g copy once; main matmuls then only wait their T
        scrap = wk_ps[0:1, 320:321]
        nc.tensor.matmul(scrap, lhsT=g_sb[:, 0:1], rhs=g_sb[:, 0:1],
                         start=True, stop=True)

        for p in range(NPAIR):
            t_p = t_tiles[p]
            g_p = g_sb[:, p::NPAIR]
            for ah in range(2):
                nc.tensor.matmul(acc[:, ah * C_OUT:(ah + 1) * C_OUT],
                                 lhsT=t_p[:, ah * 128:(ah + 1) * 128],
                                 rhs=g_p,
                                 start=False, stop=(p == NPAIR - 1),
                                 skip_group_check=True)

        # ---- store
        out_sb = work.tile([128, 2 * C_OUT], BF16)
        nc.vector.tensor_copy(out_sb, acc)
        nc.sync.dma_start(out=outp, in_=out_sb)

    _hoist_input_dmas(nc)
    return nc


def _hoist_input_dmas(nc: bass.Bass) -> None:
    """Move the three input-load DMACopys into the preamble block, ahead of
    the opening all-engine barrier. Their HWDGE prep + DGE latency then
    overlaps the barrier, so the first payload lands ~0.65us earlier.
    Consumers' semaphore waits are untouched -- the sems just fire earlier."""
    fn = nc.m.functions[0]
    blocks = fn.blocks
    pre = blocks[0]

    moved = []          # the three SP input-load DMACopys
    moved_ms = []       # the warm-up source memset (DVE)
    for blk in blocks[1:]:
        insns = blk.instructions
        keep = []
        for ins in insns:
            si = ins.sync_info
            has_wait = bool(si and si.on_wait)
            if (len(moved) < 3 and ins.opcode == "DMACopy"
                    and str(ins.engine) == "EngineType.SP" and not has_wait):
                moved.append(ins)
            elif (not moved_ms and ins.opcode == "Memset"
                    and str(ins.engine) == "EngineType.DVE"):
                moved_ms.append(ins)
            else:
                keep.append(ins)
        if len(keep) != len(insns):
            blk.instructions = keep
        if len(moved) == 3 and moved_ms:
            break
    assert len(moved) == 3, f"expected 3 input DMAs, found {len(moved)}"
    assert len(moved_ms) == 1, "expected the wsrc memset"

    # insert the DMAs at the very top of the SP stream, and push SP's
    # preamble RegisterMoves (they only zero monotonic-sem scratch
    # registers, unused by this kernel) behind the barrier join so they
    # don't delay it
    pre_insns = pre.instructions
    out = []
    sp_regmoves = []
    placed_dma = placed_ms = False
    for ins in pre_insns:
        if not placed_dma and str(ins.engine) == "EngineType.SP":
            out.extend(moved)
            placed_dma = True
        if (ins.opcode == "RegisterMove"
                and str(ins.engine) == "EngineType.SP"):
            sp_regmoves.append(ins)
            continue
        if (not placed_ms and ins.opcode == "Drain"
                and str(ins.engine) == "EngineType.DVE"):
            out.extend(moved_ms)
            placed_ms = True
        out.append(ins)
        if (ins.opcode == "EventSemaphore" and sp_regmoves
                and str(ins.engine) == "EngineType.SP"):
            out.extend(sp_regmoves)
            sp_regmoves = []
    assert placed_dma and placed_ms and not sp_regmoves
    pre.instructions = out


def shard_inputs(features, geometry, W1, b1, W2, b2) -> list[dict]:
    import ml_dtypes
    bf16 = ml_dtypes.bfloat16
    f = np.ascontiguousarray(np.asarray(features, np.float32))
    g = np.ascontiguousarray(np.asarray(geometry, np.float32))
    W1 = np.ascontiguousarray(np.asarray(W1, np.float32))
    b1 = np.ascontiguousarray(np.asarray(b1, np.float32))
    W2 = np.ascontiguousarray(np.asarray(W2, np.float32))
    b2 = np.ascontiguousarray(np.asarray(b2, np.float32))

    # wexp[j, i*64+h] = W2[h, i*32+j]
    w2r = W2.reshape(H, C_OUT, C_IN)            # [h, i, j]
    wexp = np.ascontiguousarray(
        w2r.transpose(2, 1, 0).reshape(C_IN, C_OUT * H)).astype(bf16)

    # w1bexp[(bl',x), (bl,h)] = delta(bl,bl') * [W1;b1][x, h]
    w1b = np.concatenate([W1, b1[None, :]], axis=0)      # [4, H]
    w1bexp = np.zeros((8, 128), np.float32)
    w1bexp[0:4, 0:64] = w1b
    w1bexp[4:8, 64:128] = w1b

    w1dup = np.concatenate([W1, W1], axis=1)             # [3, 128]

    b2t = np.ascontiguousarray(b2.reshape(C_OUT, C_IN).T)  # [j, i]

    maps = []
    for core in range(8):
        z, q = divmod(core, 4)
        sl = slice(q * BQ, (q + 1) * BQ)
        fq = f[z, sl]                                    # [64, j]
        gq = g[z, sl]                                    # [64, 3]

        d1 = np.zeros((D1_P, D1_W), np.float32)
        d1[0:3, 0:256] = g[z].T
        d1[0:3, 256:384] = w1dup
        # gb8[(bl'*4+x), p] = g[2p+bl', x] for x<3, 1.0 for x=3
        gb8 = np.zeros((8, NPAIR), np.float32)
        gb8[0:3, :] = gq[0::2].T
        gb8[3, :] = 1.0
        gb8[4:7, :] = gq[1::2].T
        gb8[7, :] = 1.0
        d1[0:8, 384:416] = gb8
        d1[0:8, 416:544] = w1bexp
        d1[:, 544:576] = fq[0::2].T                      # f2e [j, p]
        d1[:, 576:608] = fq[1::2].T                      # f2o
        d1[:, 608:672] = fq.T                            # fTb
        d1[:, 672:704] = b2t

        maps.append({
            "d1": d1.astype(bf16),
            "wexp": wexp,
        })
    return maps


def unshard(parts: list[np.ndarray]) -> np.ndarray:
    out = np.zeros((Z, N, C_OUT), np.float32)
    for z in range(Z):
        for q in range(4):
            p = np.asarray(parts[4 * z + q], np.float32)   # [128, 64]
            out[z, 0:128] += p[:, 0:C_OUT]
            out[z, 128:256] += p[:, C_OUT:2 * C_OUT]
    return out


def kernel(**inputs) -> np.ndarray:
    nc = build_nc(debug=False)
    in_maps = shard_inputs(**inputs)
    res = run_bass_kernel_spmd(nc, in_maps, list(range(8)))
    return unshard([r["outp"] for r in res.results])
